# revision 15
# baseline (speedup 1.0000x reference)
"""Multi-head attention (B=2, S=2048, E=1024, H=16, D=64) on 8 trn2 cores.

Sharding: core c = (b, g) with b = c // 4 (batch), g = c % 4 (head group of
4 heads = 256 features). Each core computes Q/K/V projections for its head
group, full attention for its 4 heads, and a partial output projection; a
ReduceScatter over the 4 cores of each batch sums the partials and leaves
each core with a [512, 1024] slice of the final output. The host
concatenates the slices and adds bo + bv@Wo.T (softmax rows sum to 1, so
the V bias contributes a constant row to the output -- free on host).

Precision: uniform fp16 compute (psum f32). fp8/DoubleRow was measured and
rejected: the attention here is peaky (scores/8 spans +-9), so quantization
noise on P or V does NOT average over keys; fp8 P alone costs ~2.4%% output
error and fp8 V another ~4%% against a 2e-2 gate. All matmuls cost 1 PE
cycle per output column regardless, so fp16 is as fast as bf16 with 8x the
mantissa.

Per-core dataflow (contractions on the partition dim):
  Q^T/K^T [f,s] = W-chunk.T @ x^T   (stationary = weight chunk, psum f32,
      ACT Identity copy psum->sbuf folds the per-partition bias in)
  V [k,f] = x-chunk.T @ W           (no bias: moved to host)
  S^T [k,q] = K^T-chunk.T @ Q^T     (contraction d=64)
  P = exp(S/8) via ACT -> fp16 (z in [-9.2, 8.8]: exp fits fp16 range, no
      max-subtraction or shift needed)
  O'^T [65,q] = V'-chunk.T @ P      (V' = [V | 1]: row 64 = denominator)
  O^T = O'^T[0:64] * bcast(1/denom) (fp16 reciprocal, PE outer-product
      broadcast, DVE multiply)
  Y[s,e] partial = O^T-chunk.T @ Wo^T -> psum -> sbuf -> DRAM,
      ReduceScatter(add) over each batch's 4 cores.

Engine budget per core (cost-model): PE 401k cycles (167us) dominates;
ACT ~140us (exp 128 calls + Q/K copies + yt copies); DVE ~40us (V copies,
reciprocal, bc, normalize). Emission interleaves scores/exp tiles with AV,
out-proj and next-chunk projection matmuls so the PE never starves while
exp drains the scores psum (exp 996ns vs 427ns PE per tile).
"""

import numpy as np

B, S, E, H, D = 2, 2048, 1024, 16, 64
G = 4            # head groups (tensor-parallel)
GH = H // G      # heads per group = 4
GF = GH * D      # features per group = 256
NC = 8
EC = 8           # 128-row e-chunks
QC = 4           # q chunks of 512
KB = 16          # k blocks of 128
VW = D + 1       # V' width per head (64 data + ones)

_CACHE = {}


def _build(mode: str, collective: bool, debug: bool = False):
    import concourse.bass as bass
    import concourse.mybir as mybir
    import concourse.tile as tile
    from concourse import bacc

    dt = mybir.dt
    f32 = dt.float32
    f16 = dt.float16

    nc = bacc.Bacc()

    xT = nc.dram_tensor("xT", [128, EC * S], f16, kind="ExternalInput")
    wq = nc.dram_tensor("wq", [128, EC * 256], f16, kind="ExternalInput")
    wk = nc.dram_tensor("wk", [128, EC * 256], f16, kind="ExternalInput")
    wv = nc.dram_tensor("wv", [128, EC * 256], f16, kind="ExternalInput")
    wo = nc.dram_tensor("wo", [128, 2 * E], f16, kind="ExternalInput")
    bq2 = nc.dram_tensor("bq2", [128, 2], f32, kind="ExternalInput")
    bk2 = nc.dram_tensor("bk2", [128, 2], f32, kind="ExternalInput")
    if collective:
        yout = nc.dram_tensor("yout", [S // G, E], f32, kind="ExternalOutput")
    else:
        yout = nc.dram_tensor("yout", [S, E], f32, kind="ExternalOutput")

    def emit_body(nc, tc, res):
        xT_sb = [res.tile([128, S], f16, tag=f"xT{ec}", name=f"xT{ec}")
                 for ec in range(EC)]
        wq_sb = res.tile([128, EC * 256], f16, tag="wq")
        wk_sb = res.tile([128, EC * 256], f16, tag="wk")
        wv_sb = res.tile([128, EC * 256], f16, tag="wv")
        wo_sb = res.tile([128, 2 * E], f16, tag="wo")
        QT_sb = [res.tile([128, S], f16, tag=f"QT{fb}", name=f"QT{fb}")
                 for fb in range(2)]
        KT_sb = [res.tile([128, S], f16, tag=f"KT{fb}", name=f"KT{fb}")
                 for fb in range(2)]
        V_sb = [res.tile([128, GH * VW], f16, tag=f"V{kb}", name=f"V{kb}")
                for kb in range(KB)]
        OT_sb = [[res.tile([128, 512], f16, tag=f"OT{hb}_{qc}",
                           name=f"OT{hb}_{qc}")
                  for qc in range(QC)] for hb in range(2)]
        bq_sb = res.tile([128, 2], f32, tag="bq")
        bk_sb = res.tile([128, 2], f32, tag="bk")
        on64 = res.tile([1, D], f16, tag="on64")

        nc.gpsimd.memset(on64[:], 1.0)
        for kb in range(KB):
            nc.gpsimd.memset(
                V_sb[kb][:].rearrange("p (h x) -> p h x", x=VW)[:, :, D:VW],
                1.0)

        # input DMAs: K path first so K projections start immediately
        nc.sync.dma_start(out=bk_sb[:], in_=bk2[:])
        nc.sync.dma_start(out=bq_sb[:], in_=bq2[:])
        for ec in range(EC):
            nc.sync.dma_start(out=wk_sb[:, ec * 256:(ec + 1) * 256],
                              in_=wk[:, ec * 256:(ec + 1) * 256])
        for qc in range(QC):
            for ec in range(EC):
                for hq in range(2):
                    o = qc * 512 + hq * 256
                    nc.sync.dma_start(
                        out=xT_sb[ec][:, o:o + 256],
                        in_=xT[:, ec * S + o:ec * S + o + 256])
        for ec in range(EC):
            nc.sync.dma_start(out=wq_sb[:, ec * 256:(ec + 1) * 256],
                              in_=wq[:, ec * 256:(ec + 1) * 256])
        for ec in range(EC):
            nc.sync.dma_start(out=wv_sb[:, ec * 256:(ec + 1) * 256],
                              in_=wv[:, ec * 256:(ec + 1) * 256])
        for hb in range(2):
            nc.sync.dma_start(out=wo_sb[:, hb * E:(hb + 1) * E],
                              in_=wo[:, hb * E:(hb + 1) * E])

        def emit_qk_proj(ps, w_sb, dst_sb, b_sb, fb, qc):
            # Q^T/K^T block fb (128 features = 2 heads), q chunk qc.
            pq = ps.tile([128, 512], f32, tag="pq", name="pq")
            for ec in range(EC):
                nc.tensor.matmul(
                    pq[:],
                    lhsT=w_sb[:, ec * 256 + fb * 128:ec * 256 + fb * 128 + 128],
                    rhs=xT_sb[ec][:, qc * 512:(qc + 1) * 512],
                    start=(ec == 0), stop=(ec == EC - 1))
            nc.vector.tensor_scalar(
                out=dst_sb[:, qc * 512:(qc + 1) * 512], in0=pq[:],
                scalar1=b_sb[:, fb:fb + 1], scalar2=None,
                op0=mybir.AluOpType.add)

        def emit_v_proj(ps, kb):
            # V block kb (128 k-positions), all 256 features; no bias.
            pv = ps.tile([128, GF], f32, tag="pq", name="pv")
            for ec in range(EC):
                nc.tensor.matmul(
                    pv[:],
                    lhsT=xT_sb[ec][:, kb * 128:kb * 128 + 128],
                    rhs=wv_sb[:, ec * 256:(ec + 1) * 256],
                    start=(ec == 0), stop=(ec == EC - 1))
            nc.vector.tensor_copy(
                V_sb[kb][:].rearrange("p (h x) -> p h x", x=VW)[:, :, 0:D],
                pv[:].rearrange("p (h d) -> p h d", d=D))

        def emit_scores_tile(ps, ptt, h, qc, t):
            # two k-blocks of S^T then one exp call [128, 1024] on ACT
            fb, hr = h // 2, (h % 2) * D
            pst = ps.tile([128, 1024], f32, tag="pst", name="pst")
            for kj in range(2):
                kb = 2 * t + kj
                nc.tensor.matmul(
                    pst[:, kj * 512:(kj + 1) * 512],
                    lhsT=KT_sb[fb][hr:hr + D, kb * 128:kb * 128 + 128],
                    rhs=QT_sb[fb][hr:hr + D, qc * 512:(qc + 1) * 512],
                    start=True, stop=True)
            nc.scalar.activation(
                ptt[:, t * 1024:(t + 1) * 1024], pst[:],
                mybir.ActivationFunctionType.Exp, scale=0.125)

        def emit_av(ps, h, qc, ptt):
            pot = ps.tile([128, 512], f32, tag="pot", name="pot")
            for kb in range(KB):
                nc.tensor.matmul(
                    pot[0:VW, :],
                    lhsT=V_sb[kb][:, h * VW:(h + 1) * VW],
                    rhs=ptt[:, kb * 512:(kb + 1) * 512],
                    start=(kb == 0), stop=(kb == KB - 1))
            return pot

        def emit_norm(recp, h, qc, pot):
            hb, hr = h // 2, (h % 2) * D
            rec = recp.tile([1, 512], f16, tag="rec", name="rec")
            with nc.allow_low_precision("fp16 softmax-denominator broadcast"):
                nc.vector.reciprocal(rec[:], pot[D:D + 1, :])
            nc.tensor.matmul(pot[D:D + D, :], lhsT=on64[:], rhs=rec[:],
                             start=True, stop=True)
            bc = recp.tile([D, 512], f32, tag="bc", name="bc")
            nc.vector.tensor_copy(bc[:], pot[D:D + D, :])
            nc.vector.tensor_tensor(
                out=OT_sb[hb][qc][hr:hr + D, :],
                in0=pot[0:D, :], in1=bc[:],
                op=mybir.AluOpType.mult)

        def emit_outproj_sb(ps, ysb, sb):
            qc = sb // 4
            for fc in range(2):
                pyt = ps.tile([128, 512], f32, tag="pq", name="pyt")
                for hb in range(2):
                    nc.tensor.matmul(
                        pyt[:],
                        lhsT=OT_sb[hb][qc][:, (sb % 4) * 128:
                                           (sb % 4) * 128 + 128],
                        rhs=wo_sb[:, hb * E + fc * 512:hb * E + fc * 512 + 512],
                        start=(hb == 0), stop=(hb == 1))
                yt = ysb.tile([128, 512], f32, tag="yt", name="yt")
                nc.vector.tensor_copy(yt[:], pyt[:])
                dst = y_part if collective else yout
                for dq in range(4):
                    nc.sync.dma_start(
                        out=dst[sb * 128:(sb + 1) * 128,
                                fc * 512 + dq * 128:fc * 512 + dq * 128 + 128],
                        in_=yt[:, dq * 128:(dq + 1) * 128])

        with tc.tile_pool(name="dram", bufs=1, space="DRAM") as dram, \
             tc.tile_pool(name="ps", bufs=2, space="PSUM") as ps, \
             tc.tile_pool(name="ptp", bufs=4) as ptp, \
             tc.tile_pool(name="recp", bufs=3) as recp, \
             tc.tile_pool(name="ysb", bufs=4) as ysb:
            if collective:
                y_part = dram.tile([S, E], f32, tag="ypart")
                rs_out = dram.tile([S // G, E], f32, tag="rsout")

            # K projections first (scores need all of K), then Q qc0.
            for qc in range(QC):
                for fb in range(2):
                    emit_qk_proj(ps, wk_sb, KT_sb[fb], bk_sb, fb, qc)
            for fb in range(2):
                emit_qk_proj(ps, wq_sb, QT_sb[fb], bq_sb, fb, 0)

            # Fill-work generator: small PE units woven between scores tiles
            # so the PE stays busy while ACT drains exp. Yields per unit.
            def fill_units(qc):
                # wave qc emits scores(h, qc); fill with: V proj (wave 0),
                # AV+norm of wave qc-1, outproj of wave qc-2, Q proj qc+1.
                if qc == 0:
                    for kb in range(KB):
                        yield lambda kb=kb: emit_v_proj(ps, kb)
                else:
                    for (ph, pqc, pt) in prev_ptts:
                        box = {}
                        for c4 in range(4):
                            def av_chunk(ph=ph, pt=pt, c4=c4, box=box):
                                if c4 == 0:
                                    box["pot"] = ps.tile(
                                        [128, 512], f32, tag="pot",
                                        name="pot")
                                pot = box["pot"]
                                for kb in range(c4 * 4, c4 * 4 + 4):
                                    nc.tensor.matmul(
                                        pot[0:VW, :],
                                        lhsT=V_sb[kb][:,
                                                      ph * VW:(ph + 1) * VW],
                                        rhs=pt[:, kb * 512:(kb + 1) * 512],
                                        start=(kb == 0), stop=(kb == KB - 1))
                            yield av_chunk
                        yield (lambda ph=ph, pqc=pqc, box=box:
                               emit_norm(recp, ph, pqc, box["pot"]))
                    if qc >= 2:
                        for sb in range((qc - 2) * 4, (qc - 2) * 4 + 4):
                            yield lambda sb=sb: emit_outproj_sb(ps, ysb, sb)
                if qc + 1 < QC:
                    for fb in range(2):
                        yield lambda fb=fb: emit_qk_proj(
                            ps, wq_sb, QT_sb[fb], bq_sb, fb, qc + 1)

            prev_ptts = []
            for qc in range(QC):
                fill = fill_units(qc)
                cur = []
                for h in range(GH):
                    ptt = ptp.tile([128, KB * 512], f16, tag="ptt",
                                   name="ptt")
                    for t in range(KB // 2):
                        emit_scores_tile(ps, ptt, h, qc, t)
                        u = next(fill, None)
                        if u is not None:
                            u()
                    cur.append((h, qc, ptt))
                for u in fill:
                    u()
                prev_ptts = cur

            # tail: AV+norm for qc=3, outproj for qc=2 and qc=3
            for (ph, pqc, pt) in prev_ptts:
                pot = emit_av(ps, ph, pqc, pt)
                emit_norm(recp, ph, pqc, pot)
            for sb in range(2 * 4, 4 * 4):
                emit_outproj_sb(ps, ysb, sb)

            if collective:
                nc.gpsimd.collective_compute(
                    "ReduceScatter",
                    mybir.AluOpType.add,
                    replica_groups=[[0, 1, 2, 3], [4, 5, 6, 7]],
                    ins=[y_part.opt()],
                    outs=[rs_out.opt()],
                )
                nc.sync.dma_start(out=yout[:], in_=rs_out[:])

    with tile.TileContext(nc) as tc:
        with tc.tile_pool(name="res", bufs=1) as res:
            emit_body(nc, tc, res)
    nc.finalize()
    return nc


def _in_maps(query, Wq, bq, Wk, bk, Wv, bv, Wo, bo, mode):
    F16 = np.float16

    xT16 = {}
    for b in range(B):
        x = np.asarray(query[b], np.float32)        # [S, E]
        xt = np.ascontiguousarray(x.T)              # [E, S]
        xT16[b] = np.ascontiguousarray(
            xt.reshape(EC, 128, S).transpose(1, 0, 2)
        ).reshape(128, EC * S).astype(F16)

    def w_map(Wm, g):
        # [p, (ec, fb, m)] = W[g*GF + fb*128 + m, ec*128 + p]
        Wg = np.asarray(Wm, np.float32)[g * GF:(g + 1) * GF, :]   # [256, E]
        P = Wg.reshape(2, 128, EC, 128)             # [fb, m, ec, p]
        return np.ascontiguousarray(
            P.transpose(3, 2, 0, 1)).reshape(128, EC * 256).astype(F16)

    def wo_map(g):
        Wg = np.asarray(Wo, np.float32)[:, g * GF:(g + 1) * GF]   # [E, 256]
        P = Wg.T.reshape(2, 128, E)                 # [hb, p, e]
        return np.ascontiguousarray(
            P.transpose(1, 0, 2)).reshape(128, 2 * E).astype(F16)

    maps = []
    for c in range(NC):
        b, g = c // G, c % G
        bqg = np.asarray(bq, np.float32)[g * GF:(g + 1) * GF]
        bkg = np.asarray(bk, np.float32)[g * GF:(g + 1) * GF]
        maps.append({
            "xT": xT16[b],
            "wq": w_map(Wq, g),
            "wk": w_map(Wk, g),
            "wv": w_map(Wv, g),
            "wo": wo_map(g),
            "bq2": np.ascontiguousarray(
                bqg.reshape(2, 128).T).astype(np.float32),
            "bk2": np.ascontiguousarray(
                bkg.reshape(2, 128).T).astype(np.float32),
        })
    return maps


def kernel(query, Wq, bq, Wk, bk, Wv, bv, Wo, bo,
           mode="fp16", collective=True, trace=False):
    from concourse.bass_utils import run_bass_kernel_spmd

    key = (mode, collective)
    if key not in _CACHE:
        _CACHE[key] = _build(mode, collective)
    nc = _CACHE[key]

    maps = _in_maps(query, Wq, bq, Wk, bk, Wv, bv, Wo, bo, mode)
    res = run_bass_kernel_spmd(nc, maps, list(range(NC)), trace=trace)

    out = np.empty((B, S, E), np.float32)
    if collective:
        for c in range(NC):
            b, g = c // G, c % G
            out[b, g * (S // G):(g + 1) * (S // G), :] = res.results[c]["yout"]
    else:
        for b in range(B):
            out[b] = sum(res.results[b * G + g]["yout"] for g in range(G))
    # V bias: softmax rows sum to 1, so attention(V + 1 bv^T) = attn(V) + bv,
    # and the output projection adds the constant row bv @ Wo.T.
    out += np.asarray(bo, np.float32) \
        + np.asarray(bv, np.float32) @ np.asarray(Wo, np.float32).T
    if trace:
        kernel.last_results = res
    return out


# revision 26
# speedup vs baseline: 1.2293x; 1.2293x over previous
"""Multi-head attention (B=2, S=2048, E=1024, H=16, D=64) on 8 trn2 cores.

Sharding: core c = (b, g) with b = c // 4 (batch), g = c % 4 (head group of
4 heads = 256 features). Each core computes Q/K/V projections for its head
group, full attention for its 4 heads, and a partial output projection; a
ReduceScatter over the 4 cores of each batch sums the partials and leaves
each core with a [512, 1024] slice of the final output. The host
concatenates the slices and adds bo + bv@Wo.T (softmax rows sum to 1, so
the V bias contributes a constant row to the output -- free on host).

Precision: uniform fp16 compute (psum f32). fp8/DoubleRow was measured and
rejected: the attention here is peaky (scores/8 spans +-9), so quantization
noise on P or V does NOT average over keys; fp8 P alone costs ~2.4%% output
error and fp8 V another ~4%% against a 2e-2 gate. All matmuls cost 1 PE
cycle per output column regardless, so fp16 is as fast as bf16 with 8x the
mantissa.

Per-core dataflow (contractions on the partition dim):
  Q^T/K^T [f,s] = W-chunk.T @ x^T   (stationary = weight chunk, psum f32,
      ACT Identity copy psum->sbuf folds the per-partition bias in)
  V [k,f] = x-chunk.T @ W           (no bias: moved to host)
  S^T [k,q] = K^T-chunk.T @ Q^T     (contraction d=64)
  P = exp(S/8) via ACT -> fp16 (z in [-9.2, 8.8]: exp fits fp16 range, no
      max-subtraction or shift needed)
  O'^T [65,q] = V'-chunk.T @ P      (V' = [V | 1]: row 64 = denominator)
  O^T = O'^T[0:64] * bcast(1/denom) (fp16 reciprocal, PE outer-product
      broadcast, DVE multiply)
  Y[s,e] partial = O^T-chunk.T @ Wo^T -> psum -> sbuf -> DRAM,
      ReduceScatter(add) over each batch's 4 cores.

Engine budget per core (cost-model): PE 401k cycles (167us) dominates;
ACT ~140us (exp 128 calls + Q/K copies + yt copies); DVE ~40us (V copies,
reciprocal, bc, normalize). Emission interleaves scores/exp tiles with AV,
out-proj and next-chunk projection matmuls so the PE never starves while
exp drains the scores psum (exp 996ns vs 427ns PE per tile).
"""

import numpy as np

B, S, E, H, D = 2, 2048, 1024, 16, 64
G = 4            # head groups (tensor-parallel)
GH = H // G      # heads per group = 4
GF = GH * D      # features per group = 256
NC = 8
EC = 8           # 128-row e-chunks
QC = 4           # q chunks of 512
KB = 16          # k blocks of 128
VW = D + 1       # V' width per head (64 data + ones)

_CACHE = {}


def _build(mode: str, collective: bool, debug: bool = False):
    import concourse.bass as bass
    import concourse.mybir as mybir
    import concourse.tile as tile
    from concourse import bacc

    dt = mybir.dt
    f32 = dt.float32
    f16 = dt.float16

    nc = bacc.Bacc()

    xT = nc.dram_tensor("xT", [128, EC * S], f16, kind="ExternalInput")
    wq = nc.dram_tensor("wq", [128, EC * 256], f16, kind="ExternalInput")
    wk = nc.dram_tensor("wk", [128, EC * 256], f16, kind="ExternalInput")
    wv = nc.dram_tensor("wv", [128, EC * 256], f16, kind="ExternalInput")
    wo = nc.dram_tensor("wo", [128, 2 * E], f16, kind="ExternalInput")
    bq2 = nc.dram_tensor("bq2", [128, 2], f32, kind="ExternalInput")
    bk2 = nc.dram_tensor("bk2", [128, 2], f32, kind="ExternalInput")
    if collective:
        yout = nc.dram_tensor("yout", [S // G, E], f32, kind="ExternalOutput")
    else:
        yout = nc.dram_tensor("yout", [S, E], f32, kind="ExternalOutput")

    def emit_body(nc, tc, res):
        xT_sb = [res.tile([128, S], f16, tag=f"xT{ec}", name=f"xT{ec}")
                 for ec in range(EC)]
        wq_sb = res.tile([128, EC * 256], f16, tag="wq")
        wk_sb = res.tile([128, EC * 256], f16, tag="wk")
        wv_sb = res.tile([128, EC * 256], f16, tag="wv")
        wo_sb = res.tile([128, 2 * E], f16, tag="wo")
        QT_sb = [res.tile([128, S], f16, tag=f"QT{fb}", name=f"QT{fb}")
                 for fb in range(2)]
        KT_sb = [res.tile([128, S], f16, tag=f"KT{fb}", name=f"KT{fb}")
                 for fb in range(2)]
        V_sb = [res.tile([128, GH * VW], f16, tag=f"V{kb}", name=f"V{kb}")
                for kb in range(KB)]
        OT_sb = [[res.tile([128, 512], f16, tag=f"OT{hb}_{qc}",
                           name=f"OT{hb}_{qc}")
                  for qc in range(QC)] for hb in range(2)]
        bq_sb = res.tile([128, 2], f32, tag="bq")
        bk_sb = res.tile([128, 2], f32, tag="bk")
        on64 = res.tile([1, D], f16, tag="on64")

        nc.gpsimd.memset(on64[:], 1.0)
        for kb in range(KB):
            nc.gpsimd.memset(
                V_sb[kb][:].rearrange("p (h x) -> p h x", x=VW)[:, :, D:VW],
                1.0)

        # input DMAs. Issue cost is ~0.6-1us fixed per dma_start on the
        # issuing engine, so the prologue spreads issues across SP/DVE/ACT
        # (all DMA-capable) and interleaves wk with xT-qc0 so the first K
        # projection's operands land earliest.
        issuers = [nc.sync, nc.gpsimd, nc.scalar]
        nc.sync.dma_start(out=bk_sb[:], in_=bk2[:])
        nc.sync.dma_start(out=bq_sb[:], in_=bq2[:])
        for ec in range(EC):
            issuers[ec % 3].dma_start(
                out=wk_sb[:, ec * 256:(ec + 1) * 256],
                in_=wk[:, ec * 256:(ec + 1) * 256])
            issuers[(ec + 1) % 3].dma_start(
                out=xT_sb[ec][:, 0:512], in_=xT[:, ec * S:ec * S + 512])
        for ec in range(EC):
            issuers[ec % 3].dma_start(
                out=wq_sb[:, ec * 256:(ec + 1) * 256],
                in_=wq[:, ec * 256:(ec + 1) * 256])
        for qc in range(1, QC):
            for ec in range(EC):
                issuers[(qc + ec) % 3].dma_start(
                    out=xT_sb[ec][:, qc * 512:(qc + 1) * 512],
                    in_=xT[:, ec * S + qc * 512:ec * S + qc * 512 + 512])
        for ec in range(EC):
            issuers[ec % 3].dma_start(
                out=wv_sb[:, ec * 256:(ec + 1) * 256],
                in_=wv[:, ec * 256:(ec + 1) * 256])
        for hb in range(2):
            nc.sync.dma_start(out=wo_sb[:, hb * E:(hb + 1) * E],
                              in_=wo[:, hb * E:(hb + 1) * E])

        def emit_qk_proj(ps, w_sb, dst_sb, b_sb, fb, qc):
            # Q^T/K^T block fb (128 features = 2 heads), q chunk qc.
            pq = ps.tile([128, 512], f32, tag="pq", name="pq")
            for ec in range(EC):
                nc.tensor.matmul(
                    pq[:],
                    lhsT=w_sb[:, ec * 256 + fb * 128:ec * 256 + fb * 128 + 128],
                    rhs=xT_sb[ec][:, qc * 512:(qc + 1) * 512],
                    start=(ec == 0), stop=(ec == EC - 1))
            nc.vector.tensor_scalar(
                out=dst_sb[:, qc * 512:(qc + 1) * 512], in0=pq[:],
                scalar1=b_sb[:, fb:fb + 1], scalar2=None,
                op0=mybir.AluOpType.add)

        def emit_v_proj(ps, kb):
            # V block kb (128 k-positions), all 256 features; no bias.
            pv = ps.tile([128, GF], f32, tag="pq", name="pv")
            for ec in range(EC):
                nc.tensor.matmul(
                    pv[:],
                    lhsT=xT_sb[ec][:, kb * 128:kb * 128 + 128],
                    rhs=wv_sb[:, ec * 256:(ec + 1) * 256],
                    start=(ec == 0), stop=(ec == EC - 1))
            nc.vector.tensor_copy(
                V_sb[kb][:].rearrange("p (h x) -> p h x", x=VW)[:, :, 0:D],
                pv[:].rearrange("p (h d) -> p h d", d=D))

        def emit_scores_tile(ps, ptt, h, qc, t):
            # two k-blocks of S^T then one exp call [128, 1024] on ACT
            fb, hr = h // 2, (h % 2) * D
            pst = ps.tile([128, 1024], f32, tag="pst", name="pst")
            for kj in range(2):
                kb = 2 * t + kj
                nc.tensor.matmul(
                    pst[:, kj * 512:(kj + 1) * 512],
                    lhsT=KT_sb[fb][hr:hr + D, kb * 128:kb * 128 + 128],
                    rhs=QT_sb[fb][hr:hr + D, qc * 512:(qc + 1) * 512],
                    start=True, stop=True)
            nc.scalar.activation(
                ptt[:, t * 1024:(t + 1) * 1024], pst[:],
                mybir.ActivationFunctionType.Exp, scale=0.125)

        def emit_av(ps, h, qc, ptt):
            pot = ps.tile([128, 512], f32, tag="pot", name="pot")
            for kb in range(KB):
                nc.tensor.matmul(
                    pot[0:VW, :],
                    lhsT=V_sb[kb][:, h * VW:(h + 1) * VW],
                    rhs=ptt[:, kb * 512:(kb + 1) * 512],
                    start=(kb == 0), stop=(kb == KB - 1))
            return pot

        def emit_norm(recp, h, qc, pot):
            hb, hr = h // 2, (h % 2) * D
            rec = recp.tile([1, 512], f16, tag="rec", name="rec")
            with nc.allow_low_precision("fp16 softmax-denominator broadcast"):
                nc.vector.reciprocal(rec[:], pot[D:D + 1, :])
            nc.tensor.matmul(pot[D:D + D, :], lhsT=on64[:], rhs=rec[:],
                             start=True, stop=True)
            bc = recp.tile([D, 512], f32, tag="bc", name="bc")
            nc.vector.tensor_copy(bc[:], pot[D:D + D, :])
            nc.vector.tensor_tensor(
                out=OT_sb[hb][qc][hr:hr + D, :],
                in0=pot[0:D, :], in1=bc[:],
                op=mybir.AluOpType.mult)

        def emit_outproj_sb(ps, ysb, sb):
            qc = sb // 4
            for fc in range(2):
                pyt = ps.tile([128, 512], f32, tag="pq", name="pyt")
                for hb in range(2):
                    nc.tensor.matmul(
                        pyt[:],
                        lhsT=OT_sb[hb][qc][:, (sb % 4) * 128:
                                           (sb % 4) * 128 + 128],
                        rhs=wo_sb[:, hb * E + fc * 512:hb * E + fc * 512 + 512],
                        start=(hb == 0), stop=(hb == 1))
                yt = ysb.tile([128, 512], f32, tag="yt", name="yt")
                nc.vector.tensor_copy(yt[:], pyt[:])
                dst = y_part if collective else yout
                nc.sync.dma_start(
                    out=dst[sb * 128:(sb + 1) * 128, fc * 512:(fc + 1) * 512],
                    in_=yt[:])

        with tc.tile_pool(name="dram", bufs=1, space="DRAM") as dram, \
             tc.tile_pool(name="ps", bufs=2, space="PSUM") as ps, \
             tc.tile_pool(name="ptp", bufs=4) as ptp, \
             tc.tile_pool(name="recp", bufs=3) as recp, \
             tc.tile_pool(name="ysb", bufs=4) as ysb:
            if collective:
                y_part = dram.tile([S, E], f32, tag="ypart")
                rs_out = dram.tile([S // G, E], f32, tag="rsout")

            # K projections first (scores need all of K), then Q qc0.
            for qc in range(QC):
                for fb in range(2):
                    emit_qk_proj(ps, wk_sb, KT_sb[fb], bk_sb, fb, qc)
            for fb in range(2):
                emit_qk_proj(ps, wq_sb, QT_sb[fb], bq_sb, fb, 0)

            # Fill-work generator: small PE units woven between scores tiles
            # so the PE stays busy while ACT drains exp. Yields per unit.
            def fill_units(qc):
                # wave qc emits scores(h, qc); fill with: V proj (wave 0),
                # AV+norm of wave qc-1, outproj of wave qc-2, Q proj qc+1.
                if qc == 0:
                    for kb in range(KB):
                        yield lambda kb=kb: emit_v_proj(ps, kb)
                else:
                    for (ph, pqc, pt) in prev_ptts:
                        def av_unit(ph=ph, pqc=pqc, pt=pt):
                            pot = emit_av(ps, ph, pqc, pt)
                            emit_norm(recp, ph, pqc, pot)
                        yield av_unit
                    if qc >= 2:
                        for sb in range((qc - 2) * 4, (qc - 2) * 4 + 4):
                            yield lambda sb=sb: emit_outproj_sb(ps, ysb, sb)
                if qc + 1 < QC:
                    for fb in range(2):
                        yield lambda fb=fb: emit_qk_proj(
                            ps, wq_sb, QT_sb[fb], bq_sb, fb, qc + 1)

            prev_ptts = []
            for qc in range(QC):
                fill = fill_units(qc)
                cur = []
                for h in range(GH):
                    ptt = ptp.tile([128, KB * 512], f16, tag="ptt",
                                   name="ptt")
                    for t in range(KB // 2):
                        emit_scores_tile(ps, ptt, h, qc, t)
                        u = next(fill, None)
                        if u is not None:
                            u()
                    cur.append((h, qc, ptt))
                for u in fill:
                    u()
                prev_ptts = cur

            # tail: AV for qc=3 with qc=2 out-projections woven between so
            # the PE never waits on the reciprocal/broadcast chains; the qc=3
            # out-projections (needing all four norms) come last.
            for h in range(GH):
                pot = emit_av(ps, h, 3, prev_ptts[h][2])
                emit_norm(recp, h, 3, pot)
                if h >= 1:
                    emit_outproj_sb(ps, ysb, 7 + h)
            for sb in range(11, 16):
                emit_outproj_sb(ps, ysb, sb)

            if collective:
                nc.gpsimd.collective_compute(
                    "ReduceScatter",
                    mybir.AluOpType.add,
                    replica_groups=[[0, 1, 2, 3], [4, 5, 6, 7]],
                    ins=[y_part.opt()],
                    outs=[rs_out.opt()],
                )
                nc.sync.dma_start(out=yout[:], in_=rs_out[:])

    with tile.TileContext(nc) as tc:
        with tc.tile_pool(name="res", bufs=1) as res:
            emit_body(nc, tc, res)
    nc.finalize()
    return nc


def _in_maps(query, Wq, bq, Wk, bk, Wv, bv, Wo, bo, mode):
    F16 = np.float16

    xT16 = {}
    for b in range(B):
        x = np.asarray(query[b], np.float32)        # [S, E]
        xt = np.ascontiguousarray(x.T)              # [E, S]
        xT16[b] = np.ascontiguousarray(
            xt.reshape(EC, 128, S).transpose(1, 0, 2)
        ).reshape(128, EC * S).astype(F16)

    def w_map(Wm, g):
        # [p, (ec, fb, m)] = W[g*GF + fb*128 + m, ec*128 + p]
        Wg = np.asarray(Wm, np.float32)[g * GF:(g + 1) * GF, :]   # [256, E]
        P = Wg.reshape(2, 128, EC, 128)             # [fb, m, ec, p]
        return np.ascontiguousarray(
            P.transpose(3, 2, 0, 1)).reshape(128, EC * 256).astype(F16)

    def wo_map(g):
        Wg = np.asarray(Wo, np.float32)[:, g * GF:(g + 1) * GF]   # [E, 256]
        P = Wg.T.reshape(2, 128, E)                 # [hb, p, e]
        return np.ascontiguousarray(
            P.transpose(1, 0, 2)).reshape(128, 2 * E).astype(F16)

    maps = []
    for c in range(NC):
        b, g = c // G, c % G
        bqg = np.asarray(bq, np.float32)[g * GF:(g + 1) * GF]
        bkg = np.asarray(bk, np.float32)[g * GF:(g + 1) * GF]
        maps.append({
            "xT": xT16[b],
            "wq": w_map(Wq, g),
            "wk": w_map(Wk, g),
            "wv": w_map(Wv, g),
            "wo": wo_map(g),
            "bq2": np.ascontiguousarray(
                bqg.reshape(2, 128).T).astype(np.float32),
            "bk2": np.ascontiguousarray(
                bkg.reshape(2, 128).T).astype(np.float32),
        })
    return maps


def kernel(query, Wq, bq, Wk, bk, Wv, bv, Wo, bo,
           mode="fp16", collective=True, trace=False):
    from concourse.bass_utils import run_bass_kernel_spmd

    key = (mode, collective)
    if key not in _CACHE:
        _CACHE[key] = _build(mode, collective)
    nc = _CACHE[key]

    maps = _in_maps(query, Wq, bq, Wk, bk, Wv, bv, Wo, bo, mode)
    res = run_bass_kernel_spmd(nc, maps, list(range(NC)), trace=trace)

    out = np.empty((B, S, E), np.float32)
    if collective:
        for c in range(NC):
            b, g = c // G, c % G
            out[b, g * (S // G):(g + 1) * (S // G), :] = res.results[c]["yout"]
    else:
        for b in range(B):
            out[b] = sum(res.results[b * G + g]["yout"] for g in range(G))
    # V bias: softmax rows sum to 1, so attention(V + 1 bv^T) = attn(V) + bv,
    # and the output projection adds the constant row bv @ Wo.T.
    out += np.asarray(bo, np.float32) \
        + np.asarray(bv, np.float32) @ np.asarray(Wo, np.float32).T
    if trace:
        kernel.last_results = res
    return out
